# revision 17
# baseline (speedup 1.0000x reference)
"""Trainium2 Bass kernel for nn_Decoder_22196390985918 (SPADE-style decoder).

Sharding: 8 cores = (batch b in 0..3) x (H-half in 0..1). Each core computes
out[b, :, h0:h0+64, :] for h0 = 64*(core%2).

Key algorithmic transform: the [B, 512, H, W] "middle" tensor (masked scatter
of per-region style vectors mu[b,j,:]) is never materialized. Since
middle[b,:,h,w] = mu[b, j*(h,w), :] with j* the last active region,
conv(middle) collapses to a conv over the 5 one-hot region masks sel_j with
per-batch tap tables G[j, cc, tap] = sum_k Wconv[cc, k, tap] * mu[b, j, k].
That turns ~77 GFLOP of 512-channel convs into one K=45 matmul per tile.

The SPADE branch (mask -> shared 3x3 conv -> relu -> gamma/beta 3x3 convs) is
computed directly: shared conv via K=27 im2col, gamma/beta convs as 9
accumulating K=128 taps with gamma and beta fused into one M=128 output.
The sigmoid blending factors are folded into the conv weights and biases.

All conv/table matmuls run in float32r (TF32-like); everything else is fp32.
Each im2col is built by a single multi-dim-AP DMA per output chunk; DMA
issue is spread across the sync/tensor/scalar/gpsimd queues.
"""
import os as _os

import numpy as np

import concourse.bacc as bacc
import concourse.bass as bass
import concourse.mybir as mybir
import concourse.tile as tile
from concourse.bass_utils import run_bass_kernel_spmd

dt = mybir.dt
F32 = dt.float32
F32R = dt.float32 if _os.environ.get("KF32") == "1" else dt.float32r
AF = mybir.ActivationFunctionType
ALU = mybir.AluOpType

B, C, H, W, F, L, NH = 4, 64, 128, 128, 5, 512, 128
GW = 130                    # padded grid width  (image col = grid col - 1)
SR = 66                     # seg/sel/actv grid rows (image row = h0 - 1 + r)
MR = 68                     # mask grid rows (image row = h0 - 2 + r)
SEG_N = SR * GW             # 8580
MASK_N = MR * GW            # 8840
SEG_SZ = SEG_N + 2 * GW + 2 + 520   # sel tail slack for im2col windows
MASK_SZ = MASK_N + 2 * GW + 2 + 390
ROWS = 64                   # output rows per core
NCH = 16                    # main conv chunks (4 rows x 128 cols, N=512)
ACH = 22                    # shared conv chunks (3 rows x 128 cols, N=384)
NCORES = 8


def _win_ap(base_ap, flat):
    """9-tap im2col source view: partitions from base_ap, free dims
    (ty[3] x tx[3] x flat window) as overlapping strided windows."""
    return bass.AP(tensor=base_ap.tensor, offset=base_ap.offset,
                   ap=[base_ap.ap[0], [GW, 3], [1, 3], [1, flat]])


def _build_nc():
    lvl = int(_os.environ.get("KSEC", "8"))
    nc = bacc.Bacc()

    # ---- per-core DRAM inputs -------------------------------------------
    xb = nc.dram_tensor("xb", [C, H * W], F32, kind="ExternalInput")
    xown = nc.dram_tensor("xown", [C, ROWS * W], F32, kind="ExternalInput")
    segg = nc.dram_tensor("segg", [F, SEG_N], F32, kind="ExternalInput")
    maskg = nc.dram_tensor("maskg", [3, MASK_N], F32, kind="ExternalInput")
    codes = nc.dram_tensor("codes", [F, L], F32, kind="ExternalInput")
    fcw = nc.dram_tensor("fcw", [F, L, L], F32, kind="ExternalInput")
    fcbt = nc.dram_tensor("fcbt", [L, F], F32, kind="ExternalInput")
    cgw = nc.dram_tensor("cgw", [C, L * 9], F32, kind="ExternalInput")
    cbw = nc.dram_tensor("cbw", [C, L * 9], F32, kind="ExternalInput")
    sgw = nc.dram_tensor("sgw", [C, NH * 9], F32, kind="ExternalInput")
    sbw = nc.dram_tensor("sbw", [C, NH * 9], F32, kind="ExternalInput")
    ssw = nc.dram_tensor("ssw", [NH, 27], F32, kind="ExternalInput")
    cgb = nc.dram_tensor("cgb", [C, 1], F32, kind="ExternalInput")
    cbb = nc.dram_tensor("cbb", [C, 1], F32, kind="ExternalInput")
    sgbb = nc.dram_tensor("sgbb", [C, 1], F32, kind="ExternalInput")
    sbbb = nc.dram_tensor("sbbb", [C, 1], F32, kind="ExternalInput")
    ssb = nc.dram_tensor("ssb", [NH, 1], F32, kind="ExternalInput")
    bg = nc.dram_tensor("bg", [1, 1], F32, kind="ExternalInput")
    bb = nc.dram_tensor("bb", [1, 1], F32, kind="ExternalInput")
    u5 = nc.dram_tensor("u5", [F, F], F32, kind="ExternalInput")
    ident = nc.dram_tensor("ident", [128, 128], F32, kind="ExternalInput")
    zz = nc.dram_tensor("zz", [128, 652], F32, kind="ExternalInput")
    hal = nc.dram_tensor("hal", [128, 2], F32, kind="ExternalInput")
    out_d = nc.dram_tensor("out", [C, 8, 1024], F32, kind="ExternalOutput")

    with tile.TileContext(nc) as tc:
        with (
            tc.tile_pool(name="const", bufs=1) as cst,
            tc.tile_pool(name="wcb", bufs=2) as wcbp,
            tc.tile_pool(name="wct", bufs=2) as wctp,
            tc.tile_pool(name="fcwp", bufs=3) as fcwp,
            tc.tile_pool(name="cbcp", bufs=2) as cbcp,
            tc.tile_pool(name="ttp", bufs=2) as ttp,
            tc.tile_pool(name="mimc", bufs=2) as mimcp,
            tc.tile_pool(name="simc", bufs=2) as simcp,
            tc.tile_pool(name="xs", bufs=2) as xsp,
            tc.tile_pool(name="gb", bufs=3) as gbp,
            tc.tile_pool(name="xn", bufs=2) as xnp,
            tc.tile_pool(name="ot", bufs=2) as otp,
            tc.tile_pool(name="pmain", bufs=2, space="PSUM") as pmain,
            tc.tile_pool(name="paux", bufs=2, space="PSUM") as paux,
            tc.tile_pool(name="gpsp", bufs=3, space="PSUM") as gpsp,
        ):
            # ---- constants / small tiles --------------------------------
            id_t = cst.tile([128, 128], F32)
            nc.sync.dma_start(out=id_t[:], in_=ident[:])
            id_r = cst.tile([128, 128], F32R)
            nc.sync.dma_start(out=id_r[:], in_=ident[:].bitcast(F32R))
            u5r = cst.tile([F, F], F32R)
            nc.sync.dma_start(out=u5r[:], in_=u5[:].bitcast(F32R))
            ones_t = cst.tile([128, 1], F32)
            nc.gpsimd.memset(ones_t[:], 1.0)
            eps_t = cst.tile([C, 1], F32)
            nc.gpsimd.memset(eps_t[:], 1e-5)
            half1 = cst.tile([128, 1], F32)
            nc.gpsimd.memset(half1[0:64, :], 1.0)
            nc.gpsimd.memset(half1[64:128, :], 0.0)
            zsb = cst.tile([128, 132], F32)
            nc.gpsimd.memset(zsb[:], 0.0)

            # blending factors -> gba (per-cc scale), om_gba (1 - gba)
            graw = cst.tile([128, 1], F32)
            nc.sync.dma_start(out=graw[:], in_=bg[:].to_broadcast((128, 1)))
            braw = cst.tile([128, 1], F32)
            nc.sync.dma_start(out=braw[:], in_=bb[:].to_broadcast((128, 1)))
            gsig = cst.tile([128, 1], F32)
            nc.scalar.activation(gsig[:], graw[:], AF.Sigmoid)
            bsig = cst.tile([128, 1], F32)
            nc.scalar.activation(bsig[:], braw[:], AF.Sigmoid)
            gba = cst.tile([128, 1], F32)
            nc.vector.tensor_copy(gba[0:64, :], gsig[0:64, :])
            nc.vector.tensor_copy(gba[64:128, :], bsig[64:128, :])
            om_gba = cst.tile([128, 1], F32)
            nc.scalar.activation(om_gba[:], gba[:], AF.Identity, bias=ones_t[:], scale=-1.0)

            # biases: rows 0:64 gamma, 64:128 beta
            convb = cst.tile([128, 1], F32)
            nc.sync.dma_start(out=convb[0:64, :], in_=cgb[:])
            nc.sync.dma_start(out=convb[64:128, :], in_=cbb[:])
            spadeb = cst.tile([128, 1], F32)
            nc.sync.dma_start(out=spadeb[0:64, :], in_=sgbb[:])
            nc.sync.dma_start(out=spadeb[64:128, :], in_=sbbb[:])
            ssb_t = cst.tile([NH, 1], F32)
            nc.sync.dma_start(out=ssb_t[:], in_=ssb[:])
            hal_t = cst.tile([128, 2], F32)
            nc.sync.dma_start(out=hal_t[:], in_=hal[:])

            tb1 = cst.tile([128, 1], F32)
            nc.vector.tensor_mul(tb1[:], convb[:], gba[:])
            tb2 = cst.tile([128, 1], F32)
            nc.vector.tensor_mul(tb2[:], spadeb[:], om_gba[:])
            bias_t = cst.tile([128, 1], F32)
            nc.vector.tensor_add(bias_t[:], tb1[:], tb2[:])
            bias1_t = cst.tile([128, 1], F32)
            nc.vector.tensor_add(bias1_t[:], bias_t[:], half1[:])

            # ---- instance-norm stats over the full plane ----------------
            if lvl >= 7:
                stats_t = cst.tile([C, 32, 6], F32)
                for q in range(8):
                    xt = xsp.tile([C, 4, 512], F32, tag="xs")
                    nc.scalar.dma_start(out=xt[:], in_=xb[:, q * 2048:(q + 1) * 2048]
                                        .rearrange("c (k n) -> c k n", k=4))
                    for k in range(4):
                        nc.vector.bn_stats(out=stats_t[:, 4 * q + k, :], in_=xt[:, k, :])
                mv = cst.tile([C, 2], F32)
                nc.vector.bn_aggr(out=mv[:], in_=stats_t[:])
                sd = cst.tile([C, 1], F32)
                nc.scalar.activation(sd[:], mv[:, 1:2], AF.Sqrt, bias=eps_t[:], scale=1.0)
                rstd = cst.tile([C, 1], F32)
                nc.vector.reciprocal(rstd[:], sd[:])
                nbias = cst.tile([C, 1], F32)
                nc.vector.tensor_mul(nbias[:], mv[:, 0:1], rstd[:])
                nc.vector.tensor_scalar_mul(nbias[:], nbias[:], -1.0)

            # ---- mu path: z[j,k] = sum_l fcw[j,k,l]*c[j,l]; mu = relu(z+b)
            if lvl >= 4:
                fcbt_sb = cst.tile([128, 4, F], F32)
                for kb in range(4):
                    nc.sync.dma_start(out=fcbt_sb[:, kb, :],
                                      in_=fcbt[kb * 128:(kb + 1) * 128, :])
                z_sb = cst.tile([128, 4, F], F32)
                muT = cst.tile([128, 4, F], F32R)
                for j in range(F):
                    cbc = cbcp.tile([128, L], F32, tag="cbc")
                    nc.scalar.dma_start(out=cbc[:],
                                        in_=codes[j:j + 1, :].to_broadcast((128, L)))
                    for kb in range(4):
                        fw = fcwp.tile([128, L], F32, tag="fcw")
                        nc.gpsimd.dma_start(out=fw[:], in_=fcw[j, kb * 128:(kb + 1) * 128, :])
                        tts = ttp.tile([128, L], F32, tag="tts")
                        nc.vector.tensor_mul(tts[:], fw[:], cbc[:])
                        nc.vector.reduce_sum(out=z_sb[:, kb, j:j + 1], in_=tts[:],
                                             axis=mybir.AxisListType.X)
                for kb in range(4):
                    nc.vector.tensor_add(z_sb[:, kb, :], z_sb[:, kb, :],
                                         fcbt_sb[:, kb, :])
                for kb in range(4):
                    nc.scalar.activation(muT[:, kb, :], z_sb[:, kb, :], AF.Relu)

            # ---- conv gamma/beta tap tables G -> selG (rows j*9+t) ------
            if lvl >= 5:
                gps = [gpsp.tile([F, 3, 128], F32, tag="gps", name=f"gps{_g}")
                       for _g in range(3)]
                for kb in range(4):
                    wcb = wcbp.tile([128, 1152], F32, tag="wcb")
                    nc.sync.dma_start(out=wcb[0:64, :], in_=cgw[:, kb * 1152:(kb + 1) * 1152])
                    nc.sync.dma_start(out=wcb[64:128, :], in_=cbw[:, kb * 1152:(kb + 1) * 1152])
                    nc.vector.tensor_scalar_mul(wcb[:], wcb[:], gba[:])
                    wct = wctp.tile([128, 9, 128], F32R, tag="wct")
                    wcb3 = wcb[:].rearrange("p (l t) -> p l t", t=9)
                    for t in range(9):
                        pt = paux.tile([128, 128], F32, tag="aux")
                        nc.tensor.transpose(pt[:], wcb3[:, :, t], id_t[:])
                        nc.scalar.activation(wct[:, t, :], pt[:], AF.Copy)
                    for g in range(3):
                        nc.tensor.matmul(gps[g][:], muT[:, kb, :], wct[:, 3 * g:3 * g + 3, :],
                                         start=(kb == 0), stop=(kb == 3))
                selG = cst.tile([45, 128], F32R)
                gstage = cst.tile([F, 9, 128], F32)
                for g in range(3):
                    nc.scalar.activation(gstage[:, 3 * g:3 * g + 3, :], gps[g][:], AF.Copy)
                for ty in range(3):
                    nc.sync.dma_start(
                        out=selG[15 * ty:15 * ty + 15, :],
                        in_=gstage[:, 3 * ty:3 * ty + 3, :].bitcast(F32R))

            # ---- grids ---------------------------------------------------
            # gridA: rows 0:5 seg -> overwritten in place by sel; rows 5:8 mask
            gridA = cst.tile([8, MASK_SZ], F32R)
            nc.sync.dma_start(out=gridA[0:F, 0:SEG_N], in_=segg[:].bitcast(F32R))
            nc.sync.dma_start(out=gridA[F:F + 3, 0:MASK_N], in_=maskg[:].bitcast(F32R))
            nc.sync.dma_start(out=gridA[0:8, MASK_N:MASK_SZ],
                              in_=zz[0:8, 0:MASK_SZ - MASK_N].bitcast(F32R))
            nc.sync.dma_start(out=gridA[0:F, SEG_N:SEG_N + 524],
                              in_=zz[0:F, 0:524].bitcast(F32R))

            # ---- region masks: cnt -> t -> sel (in place over seg) ------
            off = 0 if lvl >= 2 else SEG_N
            while off < SEG_N:
                n = min(512, SEG_N - off)
                pc = paux.tile([F, 512], F32, tag="aux")
                nc.tensor.matmul(pc[:, 0:n], u5r[:], gridA[0:F, off:off + n],
                                 start=True, stop=True)
                pt5 = paux.tile([F, 512], F32, tag="aux")
                nc.scalar.activation(pt5[:, 0:n], pc[:, 0:n], AF.Relu,
                                     bias=ones_t[0:F, :], scale=-1.0)
                nc.vector.tensor_mul(gridA[0:F, off:off + n],
                                     gridA[0:F, off:off + n].bitcast(F32),
                                     pt5[:, 0:n])
                off += n

            # ---- shared conv (mask 3 -> NH), K=27 im2col (rows ch*9+t) --
            if lvl >= 3:
                sswf = cst.tile([NH, 27], F32)
                nc.sync.dma_start(out=sswf[:], in_=ssw[:])
                ptp = paux.tile([27, 128], F32, tag="aux")
                nc.tensor.transpose(ptp[:], sswf[:], id_t[:])
                sswT = cst.tile([27, 128], F32R)
                nc.scalar.activation(sswT[:], ptp[:], AF.Copy)

                actv = cst.tile([NH, SR, GW], F32R)
                # zero-pad border columns (cols 0 and 129)
                bord = actv[:, :, 0:1]
                nc.vector.tensor_copy(
                    bass.AP(tensor=bord.tensor, offset=bord.offset,
                            ap=[bord.ap[0], [GW, SR], [GW - 1, 2]]),
                    zsb[:].rearrange("p (a b) -> p a b", a=SR))
                for a0 in range(0, ACH, 4):
                    ng = min(4, ACH - a0)
                    im = mimcp.tile([27, 4 * 390], F32R, tag="mimc")
                    for ty in range(3):
                        src = gridA[F:F + 3, a0 * 390 + ty * GW:]
                        src_ap = bass.AP(tensor=src.tensor, offset=src.offset,
                                         ap=[src.ap[0], [1, 3], [1, ng * 390]])
                        nc.scalar.dma_start(out=im[9 * ty:9 * ty + 9, 0:ng * 390],
                                            in_=src_ap)
                    for a in range(a0, a0 + ng):
                        r = 3 * a
                        psh = paux.tile([NH, 3, 128], F32, tag="aux")
                        rhs = im[:, (a - a0) * 390:(a - a0) * 390 + 390].rearrange(
                            "p (r c) -> p r c", r=3)[:, :, 0:128]
                        nc.tensor.matmul(psh[:], sswT[:], rhs, start=True, stop=True)
                        nc.scalar.activation(actv[:, r:r + 3, 1:129], psh[:], AF.Relu,
                                             bias=ssb_t[:], scale=1.0)
                # out-of-image actv rows are conv2d zero-padding
                nc.vector.tensor_scalar_mul(actv[:, 0, :], actv[:, 0, :].bitcast(F32),
                                            hal_t[:, 0:1])
                nc.vector.tensor_scalar_mul(actv[:, SR - 1, :], actv[:, SR - 1, :].bitcast(F32),
                                            hal_t[:, 1:2])

            # ---- spade gamma/beta lhsT ----------------------------------
            if lvl >= 6:
                sgb = cst.tile([128, 1152], F32)
                nc.sync.dma_start(out=sgb[0:64, :], in_=sgw[:])
                nc.sync.dma_start(out=sgb[64:128, :], in_=sbw[:])
                nc.vector.tensor_scalar_mul(sgb[:], sgb[:], om_gba[:])
                spT = cst.tile([128, 9, 128], F32R)
                sgb3 = sgb[:].rearrange("p (l t) -> p l t", t=9)
                for t in range(9):
                    pt = paux.tile([128, 128], F32, tag="aux")
                    nc.tensor.transpose(pt[:], sgb3[:, :, t], id_t[:])
                    nc.scalar.activation(spT[:, t, :], pt[:], AF.Copy)

            # ---- main conv + epilogue, 16 chunks of 4 rows --------------
            if lvl >= 8:
                xt2 = None
                xnt = None
                for i in range(NCH):
                    if i % 2 == 0:
                        xt2 = xnp.tile([C, 2, 4, 128], F32, tag="xn")
                        nc.gpsimd.dma_start(out=xt2[:],
                                            in_=xown[:, i * 512:(i + 2) * 512].rearrange(
                                                "c (k r w) -> c k r w", k=2, r=4))
                        xnt = otp.tile([C, 2, 4, 128], F32, tag="ot")
                    if i % 4 == 0:
                        sim = simcp.tile([45, 4 * 520], F32R, tag="simc")
                        for ty in range(3):
                            src = gridA[0:F, 4 * i * GW + ty * GW:]
                            src_ap = bass.AP(tensor=src.tensor, offset=src.offset,
                                             ap=[src.ap[0], [1, 3], [1, 4 * 520]])
                            nc.gpsimd.dma_start(out=sim[15 * ty:15 * ty + 15, :],
                                                in_=src_ap)
                    pm = pmain.tile([128, 4, 128], F32, tag="pm")
                    for t in range(9):
                        ty, tx = divmod(t, 3)
                        nc.tensor.matmul(pm[:], spT[:, t, :],
                                         actv[:, 4 * i + ty:4 * i + ty + 4, tx:tx + 128],
                                         start=(t == 0), stop=False)
                    nc.tensor.matmul(
                        pm[:], selG[:],
                        sim[:, (i % 4) * 520:(i % 4) * 520 + 520].rearrange(
                            "p (r c) -> p r c", r=4)[:, :, 0:128],
                        start=False, stop=True)

                    gb = gbp.tile([128, 4, 128], F32R, tag="gb")
                    nc.scalar.activation(gb[0:64, :, :], pm[0:64, :, :], AF.Identity,
                                         bias=bias1_t[0:64, :], scale=1.0)
                    nc.scalar.activation(gb[64:128, :, :], pm[64:128, :, :], AF.Identity,
                                         bias=bias_t[64:128, :], scale=1.0)
                    # move beta rows 64:128 down to partitions 0:64 via PE
                    pb = gpsp.tile([64, 4, 128], F32, tag="gps", name="pb")
                    nc.tensor.matmul(pb[:].rearrange("p t c -> p (t c)"), id_r[:, 64:128],
                                     gb[:].rearrange("p t c -> p (t c)"),
                                     start=True, stop=True)
                    k = i % 2
                    nc.gpsimd.tensor_scalar(xnt[:, k, :, :], xt2[:, k, :, :],
                                            rstd[:], nbias[:],
                                            op0=ALU.mult, op1=ALU.add)
                    nc.vector.tensor_mul(xnt[:, k, :, :], xnt[:, k, :, :],
                                         gb[0:64, :, :].bitcast(F32))
                    nc.vector.tensor_add(xnt[:, k, :, :].rearrange("p t c -> p (t c)"),
                                         xnt[:, k, :, :].rearrange("p t c -> p (t c)"),
                                         pb[:].rearrange("p t c -> p (t c)"))
                    if i % 2 == 1:
                        nc.sync.dma_start(out=out_d[:, i // 2, :],
                                          in_=xnt[:].rearrange("c k r w -> c (k r w)"))

    nc.finalize()
    return nc


_NC = None


def kernel(**inputs):
    global _NC
    x = np.asarray(inputs["x"], dtype=np.float32)
    segmap = np.asarray(inputs["segmap"], dtype=np.float32)
    codes_vector = np.asarray(inputs["codes_vector"], dtype=np.float32)
    mask = np.asarray(inputs["mask"], dtype=np.float32)
    fc_w = np.ascontiguousarray(np.asarray(inputs["fc_w"], dtype=np.float32))
    fc_b = np.asarray(inputs["fc_b"], dtype=np.float32)
    conv_gamma_w = np.asarray(inputs["conv_gamma_w"], dtype=np.float32)
    conv_gamma_b = np.asarray(inputs["conv_gamma_b"], dtype=np.float32)
    conv_beta_w = np.asarray(inputs["conv_beta_w"], dtype=np.float32)
    conv_beta_b = np.asarray(inputs["conv_beta_b"], dtype=np.float32)
    spade_shared_w = np.asarray(inputs["spade_shared_w"], dtype=np.float32)
    spade_shared_b = np.asarray(inputs["spade_shared_b"], dtype=np.float32)
    spade_gamma_w = np.asarray(inputs["spade_gamma_w"], dtype=np.float32)
    spade_gamma_b = np.asarray(inputs["spade_gamma_b"], dtype=np.float32)
    spade_beta_w = np.asarray(inputs["spade_beta_w"], dtype=np.float32)
    spade_beta_b = np.asarray(inputs["spade_beta_b"], dtype=np.float32)
    blending_gamma = np.asarray(inputs["blending_gamma"], dtype=np.float32)
    blending_beta = np.asarray(inputs["blending_beta"], dtype=np.float32)

    if _NC is None:
        _NC = _build_nc()

    shared = {
        "fcw": np.ascontiguousarray(fc_w),
        "fcbt": np.ascontiguousarray(fc_b.T),
        "cgw": np.ascontiguousarray(conv_gamma_w.reshape(C, L * 9)),
        "cbw": np.ascontiguousarray(conv_beta_w.reshape(C, L * 9)),
        "sgw": np.ascontiguousarray(spade_gamma_w.reshape(C, NH * 9)),
        "sbw": np.ascontiguousarray(spade_beta_w.reshape(C, NH * 9)),
        "ssw": np.ascontiguousarray(spade_shared_w.transpose(0, 2, 1, 3).reshape(NH, 27)),
        "cgb": conv_gamma_b.reshape(C, 1), "cbb": conv_beta_b.reshape(C, 1),
        "sgbb": spade_gamma_b.reshape(C, 1), "sbbb": spade_beta_b.reshape(C, 1),
        "ssb": spade_shared_b.reshape(NH, 1),
        "bg": blending_gamma.reshape(1, 1), "bb": blending_beta.reshape(1, 1),
        "u5": np.tril(np.ones((F, F), np.float32), -1),
        "ident": np.eye(128, dtype=np.float32),
        "zz": np.zeros((128, 652), np.float32),
    }

    in_maps = []
    for c in range(NCORES):
        b, half = divmod(c, 2)
        h0 = half * ROWS
        segp = np.zeros((F, SR, GW), np.float32)
        r_lo, r_hi = h0 - 1, h0 + ROWS + 1  # exclusive
        s_lo, s_hi = max(r_lo, 0), min(r_hi, H)
        segp[:, s_lo - r_lo:s_hi - r_lo, 1:129] = segmap[b, :, s_lo:s_hi, :]
        maskp = np.zeros((3, MR, GW), np.float32)
        m_lo, m_hi = h0 - 2, h0 + ROWS + 2
        ms_lo, ms_hi = max(m_lo, 0), min(m_hi, H)
        maskp[:, ms_lo - m_lo:ms_hi - m_lo, 1:129] = mask[b, :, ms_lo:ms_hi, :]
        in_maps.append(dict(
            shared,
            xb=np.ascontiguousarray(x[b].reshape(C, H * W)),
            xown=np.ascontiguousarray(x[b, :, h0:h0 + ROWS, :].reshape(C, ROWS * W)),
            hal=np.ones((128, 2), np.float32) * np.array(
                [0.0 if h0 == 0 else 1.0, 0.0 if h0 + ROWS == H else 1.0],
                np.float32)[None, :],
            segg=np.ascontiguousarray(segp.reshape(F, SEG_N)),
            maskg=np.ascontiguousarray(maskp.reshape(3, MASK_N)),
            codes=np.ascontiguousarray(codes_vector[b]),
        ))

    res = run_bass_kernel_spmd(_NC, in_maps, list(range(NCORES)))

    out = np.empty((B, C, H, W), np.float32)
    for c in range(NCORES):
        b, half = divmod(c, 2)
        h0 = half * ROWS
        out[b, :, h0:h0 + ROWS, :] = res.results[c]["out"].reshape(C, ROWS, W)
    return out


# revision 18
# speedup vs baseline: 1.0084x; 1.0084x over previous
"""Trainium2 Bass kernel for nn_Decoder_22196390985918 (SPADE-style decoder).

Sharding: 8 cores = (batch b in 0..3) x (H-half in 0..1). Each core computes
out[b, :, h0:h0+64, :] for h0 = 64*(core%2).

Key algorithmic transform: the [B, 512, H, W] "middle" tensor (masked scatter
of per-region style vectors mu[b,j,:]) is never materialized. Since
middle[b,:,h,w] = mu[b, j*(h,w), :] with j* the last active region,
conv(middle) collapses to a conv over the 5 one-hot region masks sel_j with
per-batch tap tables G[j, cc, tap] = sum_k Wconv[cc, k, tap] * mu[b, j, k].
That turns ~77 GFLOP of 512-channel convs into one K=45 matmul per tile.

The SPADE branch (mask -> shared 3x3 conv -> relu -> gamma/beta 3x3 convs) is
computed directly: shared conv via K=27 im2col, gamma/beta convs as 9
accumulating K=128 taps with gamma and beta fused into one M=128 output.
The sigmoid blending factors are folded into the conv weights and biases.

All conv/table matmuls run in float32r (TF32-like); everything else is fp32.
Each im2col is built by a single multi-dim-AP DMA per output chunk; DMA
issue is spread across the sync/tensor/scalar/gpsimd queues.
"""
import os as _os

import numpy as np

import concourse.bacc as bacc
import concourse.bass as bass
import concourse.mybir as mybir
import concourse.tile as tile
from concourse.bass_utils import run_bass_kernel_spmd

dt = mybir.dt
F32 = dt.float32
F32R = dt.float32 if _os.environ.get("KF32") == "1" else dt.float32r
AF = mybir.ActivationFunctionType
ALU = mybir.AluOpType

B, C, H, W, F, L, NH = 4, 64, 128, 128, 5, 512, 128
GW = 130                    # padded grid width  (image col = grid col - 1)
SR = 66                     # seg/sel/actv grid rows (image row = h0 - 1 + r)
MR = 68                     # mask grid rows (image row = h0 - 2 + r)
SEG_N = SR * GW             # 8580
MASK_N = MR * GW            # 8840
SEG_SZ = SEG_N + 2 * GW + 2 + 520   # sel tail slack for im2col windows
MASK_SZ = MASK_N + 2 * GW + 2 + 390
ROWS = 64                   # output rows per core
NCH = 16                    # main conv chunks (4 rows x 128 cols, N=512)
ACH = 22                    # shared conv chunks (3 rows x 128 cols, N=384)
NCORES = 8


def _win_ap(base_ap, flat):
    """9-tap im2col source view: partitions from base_ap, free dims
    (ty[3] x tx[3] x flat window) as overlapping strided windows."""
    return bass.AP(tensor=base_ap.tensor, offset=base_ap.offset,
                   ap=[base_ap.ap[0], [GW, 3], [1, 3], [1, flat]])


def _build_nc():
    lvl = int(_os.environ.get("KSEC", "8"))
    nc = bacc.Bacc()

    # ---- per-core DRAM inputs -------------------------------------------
    xb = nc.dram_tensor("xb", [C, H * W], F32, kind="ExternalInput")
    xown = nc.dram_tensor("xown", [C, ROWS * W], F32, kind="ExternalInput")
    segg = nc.dram_tensor("segg", [F, SEG_N + 264], F32, kind="ExternalInput")
    maskg = nc.dram_tensor("maskg", [3, MASK_N + 264], F32, kind="ExternalInput")
    codes = nc.dram_tensor("codes", [F, L], F32, kind="ExternalInput")
    fcw = nc.dram_tensor("fcw", [F, L, L], F32, kind="ExternalInput")
    fcbt = nc.dram_tensor("fcbt", [L, F], F32, kind="ExternalInput")
    cgw = nc.dram_tensor("cgw", [C, L * 9], F32, kind="ExternalInput")
    cbw = nc.dram_tensor("cbw", [C, L * 9], F32, kind="ExternalInput")
    sgw = nc.dram_tensor("sgw", [C, NH * 9], F32, kind="ExternalInput")
    sbw = nc.dram_tensor("sbw", [C, NH * 9], F32, kind="ExternalInput")
    ssw = nc.dram_tensor("ssw", [NH, 27], F32, kind="ExternalInput")
    cgb = nc.dram_tensor("cgb", [C, 1], F32, kind="ExternalInput")
    cbb = nc.dram_tensor("cbb", [C, 1], F32, kind="ExternalInput")
    sgbb = nc.dram_tensor("sgbb", [C, 1], F32, kind="ExternalInput")
    sbbb = nc.dram_tensor("sbbb", [C, 1], F32, kind="ExternalInput")
    ssb = nc.dram_tensor("ssb", [NH, 1], F32, kind="ExternalInput")
    bg = nc.dram_tensor("bg", [1, 1], F32, kind="ExternalInput")
    bb = nc.dram_tensor("bb", [1, 1], F32, kind="ExternalInput")
    u5 = nc.dram_tensor("u5", [45, 45], F32, kind="ExternalInput")
    ident = nc.dram_tensor("ident", [128, 128], F32, kind="ExternalInput")
    zz = nc.dram_tensor("zz", [128, 652], F32, kind="ExternalInput")
    hal = nc.dram_tensor("hal", [128, 2], F32, kind="ExternalInput")
    out_d = nc.dram_tensor("out", [C, 8, 1024], F32, kind="ExternalOutput")

    with tile.TileContext(nc) as tc:
        with (
            tc.tile_pool(name="const", bufs=1) as cst,
            tc.tile_pool(name="wcb", bufs=2) as wcbp,
            tc.tile_pool(name="wct", bufs=2) as wctp,
            tc.tile_pool(name="fcwp", bufs=3) as fcwp,
            tc.tile_pool(name="cbcp", bufs=2) as cbcp,
            tc.tile_pool(name="ttp", bufs=2) as ttp,
            tc.tile_pool(name="xs", bufs=2) as xsp,
            tc.tile_pool(name="gb", bufs=3) as gbp,
            tc.tile_pool(name="xn", bufs=2) as xnp,
            tc.tile_pool(name="ot", bufs=2) as otp,
            tc.tile_pool(name="pmain", bufs=2, space="PSUM") as pmain,
            tc.tile_pool(name="paux", bufs=2, space="PSUM") as paux,
            tc.tile_pool(name="gpsp", bufs=3, space="PSUM") as gpsp,
        ):
            # ---- constants / small tiles --------------------------------
            id_t = cst.tile([128, 128], F32)
            nc.sync.dma_start(out=id_t[:], in_=ident[:])
            id_r = cst.tile([128, 128], F32R)
            nc.sync.dma_start(out=id_r[:], in_=ident[:].bitcast(F32R))
            u5r = cst.tile([45, 45], F32R)
            nc.sync.dma_start(out=u5r[:], in_=u5[:].bitcast(F32R))
            ones_t = cst.tile([128, 1], F32)
            nc.gpsimd.memset(ones_t[:], 1.0)
            eps_t = cst.tile([C, 1], F32)
            nc.gpsimd.memset(eps_t[:], 1e-5)
            half1 = cst.tile([128, 1], F32)
            nc.gpsimd.memset(half1[0:64, :], 1.0)
            nc.gpsimd.memset(half1[64:128, :], 0.0)
            zsb = cst.tile([128, 132], F32)
            nc.gpsimd.memset(zsb[:], 0.0)

            # blending factors -> gba (per-cc scale), om_gba (1 - gba)
            graw = cst.tile([128, 1], F32)
            nc.sync.dma_start(out=graw[:], in_=bg[:].to_broadcast((128, 1)))
            braw = cst.tile([128, 1], F32)
            nc.sync.dma_start(out=braw[:], in_=bb[:].to_broadcast((128, 1)))
            gsig = cst.tile([128, 1], F32)
            nc.scalar.activation(gsig[:], graw[:], AF.Sigmoid)
            bsig = cst.tile([128, 1], F32)
            nc.scalar.activation(bsig[:], braw[:], AF.Sigmoid)
            gba = cst.tile([128, 1], F32)
            nc.vector.tensor_copy(gba[0:64, :], gsig[0:64, :])
            nc.vector.tensor_copy(gba[64:128, :], bsig[64:128, :])
            om_gba = cst.tile([128, 1], F32)
            nc.scalar.activation(om_gba[:], gba[:], AF.Identity, bias=ones_t[:], scale=-1.0)

            # biases: rows 0:64 gamma, 64:128 beta
            convb = cst.tile([128, 1], F32)
            nc.sync.dma_start(out=convb[0:64, :], in_=cgb[:])
            nc.sync.dma_start(out=convb[64:128, :], in_=cbb[:])
            spadeb = cst.tile([128, 1], F32)
            nc.sync.dma_start(out=spadeb[0:64, :], in_=sgbb[:])
            nc.sync.dma_start(out=spadeb[64:128, :], in_=sbbb[:])
            ssb_t = cst.tile([NH, 1], F32)
            nc.sync.dma_start(out=ssb_t[:], in_=ssb[:])
            hal_t = cst.tile([128, 2], F32)
            nc.sync.dma_start(out=hal_t[:], in_=hal[:])

            tb1 = cst.tile([128, 1], F32)
            nc.vector.tensor_mul(tb1[:], convb[:], gba[:])
            tb2 = cst.tile([128, 1], F32)
            nc.vector.tensor_mul(tb2[:], spadeb[:], om_gba[:])
            bias_t = cst.tile([128, 1], F32)
            nc.vector.tensor_add(bias_t[:], tb1[:], tb2[:])
            bias1_t = cst.tile([128, 1], F32)
            nc.vector.tensor_add(bias1_t[:], bias_t[:], half1[:])

            # ---- instance-norm stats over the full plane ----------------
            if lvl >= 7:
                stats_t = cst.tile([C, 32, 6], F32)
                for q in range(8):
                    xt = xsp.tile([C, 4, 512], F32, tag="xs")
                    nc.scalar.dma_start(out=xt[:], in_=xb[:, q * 2048:(q + 1) * 2048]
                                        .rearrange("c (k n) -> c k n", k=4))
                    for k in range(4):
                        nc.vector.bn_stats(out=stats_t[:, 4 * q + k, :], in_=xt[:, k, :])
                mv = cst.tile([C, 2], F32)
                nc.vector.bn_aggr(out=mv[:], in_=stats_t[:])
                sd = cst.tile([C, 1], F32)
                nc.scalar.activation(sd[:], mv[:, 1:2], AF.Sqrt, bias=eps_t[:], scale=1.0)
                rstd = cst.tile([C, 1], F32)
                nc.vector.reciprocal(rstd[:], sd[:])
                nbias = cst.tile([C, 1], F32)
                nc.vector.tensor_mul(nbias[:], mv[:, 0:1], rstd[:])
                nc.vector.tensor_scalar_mul(nbias[:], nbias[:], -1.0)

            # ---- mu path: z[j,k] = sum_l fcw[j,k,l]*c[j,l]; mu = relu(z+b)
            if lvl >= 4:
                fcbt_sb = cst.tile([128, 4, F], F32)
                for kb in range(4):
                    nc.sync.dma_start(out=fcbt_sb[:, kb, :],
                                      in_=fcbt[kb * 128:(kb + 1) * 128, :])
                z_sb = cst.tile([128, 4, F], F32)
                muT = cst.tile([128, 4, F], F32R)
                for j in range(F):
                    cbc = cbcp.tile([128, L], F32, tag="cbc")
                    nc.scalar.dma_start(out=cbc[:],
                                        in_=codes[j:j + 1, :].to_broadcast((128, L)))
                    for kb in range(4):
                        fw = fcwp.tile([128, L], F32, tag="fcw")
                        nc.gpsimd.dma_start(out=fw[:], in_=fcw[j, kb * 128:(kb + 1) * 128, :])
                        tts = ttp.tile([128, L], F32, tag="tts")
                        nc.vector.tensor_mul(tts[:], fw[:], cbc[:])
                        nc.vector.reduce_sum(out=z_sb[:, kb, j:j + 1], in_=tts[:],
                                             axis=mybir.AxisListType.X)
                for kb in range(4):
                    nc.vector.tensor_add(z_sb[:, kb, :], z_sb[:, kb, :],
                                         fcbt_sb[:, kb, :])
                for kb in range(4):
                    nc.scalar.activation(muT[:, kb, :], z_sb[:, kb, :], AF.Relu)

            # ---- conv gamma/beta tap tables G -> selG (rows j*9+t) ------
            if lvl >= 5:
                gps = [gpsp.tile([F, 3, 128], F32, tag="gps", name=f"gps{_g}")
                       for _g in range(3)]
                for kb in range(4):
                    wcb = wcbp.tile([128, 1152], F32, tag="wcb")
                    nc.sync.dma_start(out=wcb[0:64, :], in_=cgw[:, kb * 1152:(kb + 1) * 1152])
                    nc.sync.dma_start(out=wcb[64:128, :], in_=cbw[:, kb * 1152:(kb + 1) * 1152])
                    nc.vector.tensor_scalar_mul(wcb[:], wcb[:], gba[:])
                    wct = wctp.tile([128, 9, 128], F32R, tag="wct")
                    wcb3 = wcb[:].rearrange("p (l t) -> p l t", t=9)
                    for t in range(9):
                        pt = paux.tile([128, 128], F32, tag="aux")
                        nc.tensor.transpose(pt[:], wcb3[:, :, t], id_t[:])
                        nc.scalar.activation(wct[:, t, :], pt[:], AF.Copy)
                    for g in range(3):
                        nc.tensor.matmul(gps[g][:], muT[:, kb, :], wct[:, 3 * g:3 * g + 3, :],
                                         start=(kb == 0), stop=(kb == 3))
                selG = cst.tile([45, 128], F32R)
                gstage = cst.tile([F, 9, 128], F32)
                for g in range(3):
                    nc.scalar.activation(gstage[:, 3 * g:3 * g + 3, :], gps[g][:], AF.Copy)
                for t in range(9):
                    nc.sync.dma_start(out=selG[F * t:F * t + F, :],
                                      in_=gstage[:, t, :].bitcast(F32R))

            # ---- grids: pre-shifted replicated loads --------------------
            # sel45 rows (r=(ty,tx), j) = seg_j shifted by ty*GW+tx; sel
            # overwrites seg in place. mask27 rows (r, ch) likewise.
            sel45 = cst.tile([45, SEG_N], F32R)
            segp = segg[:].ap[0][0]
            for ty in range(3):
                src = bass.AP(tensor=segg[:].tensor, offset=ty * GW,
                              ap=[[1, 3], [segp, F], [1, SEG_N]])
                nc.sync.dma_start(out=sel45[15 * ty:15 * ty + 15, :],
                                  in_=src.bitcast(F32R))
            mask27 = cst.tile([27, MASK_N], F32R)
            maskp_ = maskg[:].ap[0][0]
            for ty in range(3):
                src = bass.AP(tensor=maskg[:].tensor, offset=ty * GW,
                              ap=[[1, 3], [maskp_, 3], [1, MASK_N]])
                nc.scalar.dma_start(out=mask27[9 * ty:9 * ty + 9, :],
                                    in_=src.bitcast(F32R))

            # ---- region masks: cnt -> t -> sel (in place over seg) ------
            off = 0 if lvl >= 2 else SEG_N
            while off < SEG_N:
                n = min(512, SEG_N - off)
                pc = paux.tile([45, 512], F32, tag="aux")
                nc.tensor.matmul(pc[:, 0:n], u5r[:], sel45[:, off:off + n],
                                 start=True, stop=True)
                pt5 = paux.tile([45, 512], F32, tag="aux")
                nc.scalar.activation(pt5[:, 0:n], pc[:, 0:n], AF.Relu,
                                     bias=ones_t[0:45, :], scale=-1.0)
                nc.vector.tensor_mul(sel45[:, off:off + n],
                                     sel45[:, off:off + n].bitcast(F32),
                                     pt5[:, 0:n])
                off += n

            # ---- shared conv (mask 3 -> NH), K=27 im2col (rows ch*9+t) --
            if lvl >= 3:
                sswf = cst.tile([NH, 27], F32)
                nc.sync.dma_start(out=sswf[:], in_=ssw[:])
                ptp = paux.tile([27, 128], F32, tag="aux")
                nc.tensor.transpose(ptp[:], sswf[:], id_t[:])
                sswT = cst.tile([27, 128], F32R)
                nc.scalar.activation(sswT[:], ptp[:], AF.Copy)

                actv = cst.tile([NH, SR, GW], F32R)
                # zero-pad border columns (cols 0 and 129)
                bord = actv[:, :, 0:1]
                nc.vector.tensor_copy(
                    bass.AP(tensor=bord.tensor, offset=bord.offset,
                            ap=[bord.ap[0], [GW, SR], [GW - 1, 2]]),
                    zsb[:].rearrange("p (a b) -> p a b", a=SR))
                m3 = mask27[:].rearrange("p (r c) -> p r c", c=GW)
                for a in range(ACH):
                    r = 3 * a
                    psh = paux.tile([NH, 3, 128], F32, tag="aux")
                    nc.tensor.matmul(psh[:], sswT[:], m3[:, r:r + 3, 0:128],
                                     start=True, stop=True)
                    nc.scalar.activation(actv[:, r:r + 3, 1:129], psh[:], AF.Relu,
                                         bias=ssb_t[:], scale=1.0)
                # out-of-image actv rows are conv2d zero-padding
                nc.vector.tensor_scalar_mul(actv[:, 0, :], actv[:, 0, :].bitcast(F32),
                                            hal_t[:, 0:1])
                nc.vector.tensor_scalar_mul(actv[:, SR - 1, :], actv[:, SR - 1, :].bitcast(F32),
                                            hal_t[:, 1:2])

            # ---- spade gamma/beta lhsT ----------------------------------
            if lvl >= 6:
                sgb = cst.tile([128, 1152], F32)
                nc.sync.dma_start(out=sgb[0:64, :], in_=sgw[:])
                nc.sync.dma_start(out=sgb[64:128, :], in_=sbw[:])
                nc.vector.tensor_scalar_mul(sgb[:], sgb[:], om_gba[:])
                spT = cst.tile([128, 9, 128], F32R)
                sgb3 = sgb[:].rearrange("p (l t) -> p l t", t=9)
                for t in range(9):
                    pt = paux.tile([128, 128], F32, tag="aux")
                    nc.tensor.transpose(pt[:], sgb3[:, :, t], id_t[:])
                    nc.scalar.activation(spT[:, t, :], pt[:], AF.Copy)

            # ---- main conv + epilogue, 16 chunks of 4 rows --------------
            if lvl >= 8:
                xt2 = None
                xnt = None
                for i in range(NCH):
                    if i % 2 == 0:
                        xt2 = xnp.tile([C, 2, 4, 128], F32, tag="xn")
                        nc.gpsimd.dma_start(out=xt2[:],
                                            in_=xown[:, i * 512:(i + 2) * 512].rearrange(
                                                "c (k r w) -> c k r w", k=2, r=4))
                        xnt = otp.tile([C, 2, 4, 128], F32, tag="ot")
                    pm = pmain.tile([128, 4, 128], F32, tag="pm")
                    for t in range(9):
                        ty, tx = divmod(t, 3)
                        nc.tensor.matmul(pm[:], spT[:, t, :],
                                         actv[:, 4 * i + ty:4 * i + ty + 4, tx:tx + 128],
                                         start=(t == 0), stop=False)
                    s3 = sel45[:].rearrange("p (r c) -> p r c", c=GW)
                    nc.tensor.matmul(pm[:], selG[:], s3[:, 4 * i:4 * i + 4, 0:128],
                                     start=False, stop=True)

                    gb = gbp.tile([128, 4, 128], F32R, tag="gb")
                    nc.scalar.activation(gb[0:64, :, :], pm[0:64, :, :], AF.Identity,
                                         bias=bias1_t[0:64, :], scale=1.0)
                    nc.scalar.activation(gb[64:128, :, :], pm[64:128, :, :], AF.Identity,
                                         bias=bias_t[64:128, :], scale=1.0)
                    # move beta rows 64:128 down to partitions 0:64 via PE
                    pb = gpsp.tile([64, 4, 128], F32, tag="gps", name="pb")
                    nc.tensor.matmul(pb[:].rearrange("p t c -> p (t c)"), id_r[:, 64:128],
                                     gb[:].rearrange("p t c -> p (t c)"),
                                     start=True, stop=True)
                    k = i % 2
                    nc.gpsimd.tensor_scalar(xnt[:, k, :, :], xt2[:, k, :, :],
                                            rstd[:], nbias[:],
                                            op0=ALU.mult, op1=ALU.add)
                    nc.vector.tensor_mul(xnt[:, k, :, :], xnt[:, k, :, :],
                                         gb[0:64, :, :].bitcast(F32))
                    nc.vector.tensor_add(xnt[:, k, :, :].rearrange("p t c -> p (t c)"),
                                         xnt[:, k, :, :].rearrange("p t c -> p (t c)"),
                                         pb[:].rearrange("p t c -> p (t c)"))
                    if i % 2 == 1:
                        nc.sync.dma_start(out=out_d[:, i // 2, :],
                                          in_=xnt[:].rearrange("c k r w -> c (k r w)"))

    nc.finalize()
    return nc


_NC = None


def kernel(**inputs):
    global _NC
    x = np.asarray(inputs["x"], dtype=np.float32)
    segmap = np.asarray(inputs["segmap"], dtype=np.float32)
    codes_vector = np.asarray(inputs["codes_vector"], dtype=np.float32)
    mask = np.asarray(inputs["mask"], dtype=np.float32)
    fc_w = np.ascontiguousarray(np.asarray(inputs["fc_w"], dtype=np.float32))
    fc_b = np.asarray(inputs["fc_b"], dtype=np.float32)
    conv_gamma_w = np.asarray(inputs["conv_gamma_w"], dtype=np.float32)
    conv_gamma_b = np.asarray(inputs["conv_gamma_b"], dtype=np.float32)
    conv_beta_w = np.asarray(inputs["conv_beta_w"], dtype=np.float32)
    conv_beta_b = np.asarray(inputs["conv_beta_b"], dtype=np.float32)
    spade_shared_w = np.asarray(inputs["spade_shared_w"], dtype=np.float32)
    spade_shared_b = np.asarray(inputs["spade_shared_b"], dtype=np.float32)
    spade_gamma_w = np.asarray(inputs["spade_gamma_w"], dtype=np.float32)
    spade_gamma_b = np.asarray(inputs["spade_gamma_b"], dtype=np.float32)
    spade_beta_w = np.asarray(inputs["spade_beta_w"], dtype=np.float32)
    spade_beta_b = np.asarray(inputs["spade_beta_b"], dtype=np.float32)
    blending_gamma = np.asarray(inputs["blending_gamma"], dtype=np.float32)
    blending_beta = np.asarray(inputs["blending_beta"], dtype=np.float32)

    if _NC is None:
        _NC = _build_nc()

    shared = {
        "fcw": np.ascontiguousarray(fc_w),
        "fcbt": np.ascontiguousarray(fc_b.T),
        "cgw": np.ascontiguousarray(conv_gamma_w.reshape(C, L * 9)),
        "cbw": np.ascontiguousarray(conv_beta_w.reshape(C, L * 9)),
        "sgw": np.ascontiguousarray(spade_gamma_w.reshape(C, NH * 9)),
        "sbw": np.ascontiguousarray(spade_beta_w.reshape(C, NH * 9)),
        "ssw": np.ascontiguousarray(spade_shared_w.transpose(0, 2, 3, 1).reshape(NH, 27)),
        "cgb": conv_gamma_b.reshape(C, 1), "cbb": conv_beta_b.reshape(C, 1),
        "sgbb": spade_gamma_b.reshape(C, 1), "sbbb": spade_beta_b.reshape(C, 1),
        "ssb": spade_shared_b.reshape(NH, 1),
        "bg": blending_gamma.reshape(1, 1), "bb": blending_beta.reshape(1, 1),
        "u5": np.kron(np.eye(9, dtype=np.float32), np.tril(np.ones((F, F), np.float32), -1)),
        "ident": np.eye(128, dtype=np.float32),
        "zz": np.zeros((128, 652), np.float32),
    }

    in_maps = []
    for c in range(NCORES):
        b, half = divmod(c, 2)
        h0 = half * ROWS
        segp = np.zeros((F, SR * GW + 264), np.float32).reshape(F, -1)
        segp2 = np.zeros((F, SR, GW), np.float32)
        r_lo, r_hi = h0 - 1, h0 + ROWS + 1  # exclusive
        s_lo, s_hi = max(r_lo, 0), min(r_hi, H)
        segp2[:, s_lo - r_lo:s_hi - r_lo, 1:129] = segmap[b, :, s_lo:s_hi, :]
        segp[:, 0:SR * GW] = segp2.reshape(F, -1)
        maskp = np.zeros((3, MR * GW + 264), np.float32)
        maskp2 = np.zeros((3, MR, GW), np.float32)
        m_lo, m_hi = h0 - 2, h0 + ROWS + 2
        ms_lo, ms_hi = max(m_lo, 0), min(m_hi, H)
        maskp2[:, ms_lo - m_lo:ms_hi - m_lo, 1:129] = mask[b, :, ms_lo:ms_hi, :]
        maskp[:, 0:MR * GW] = maskp2.reshape(3, -1)
        in_maps.append(dict(
            shared,
            xb=np.ascontiguousarray(x[b].reshape(C, H * W)),
            xown=np.ascontiguousarray(x[b, :, h0:h0 + ROWS, :].reshape(C, ROWS * W)),
            hal=np.ones((128, 2), np.float32) * np.array(
                [0.0 if h0 == 0 else 1.0, 0.0 if h0 + ROWS == H else 1.0],
                np.float32)[None, :],
            segg=np.ascontiguousarray(segp),
            maskg=np.ascontiguousarray(maskp),
            codes=np.ascontiguousarray(codes_vector[b]),
        ))

    res = run_bass_kernel_spmd(_NC, in_maps, list(range(NCORES)))

    out = np.empty((B, C, H, W), np.float32)
    for c in range(NCORES):
        b, half = divmod(c, 2)
        h0 = half * ROWS
        out[b, :, h0:h0 + ROWS, :] = res.results[c]["out"].reshape(C, ROWS, W)
    return out


# revision 20
# speedup vs baseline: 1.0708x; 1.0619x over previous
"""Trainium2 Bass kernel for nn_Decoder_22196390985918 (SPADE-style decoder).

Sharding: 8 cores = (batch b in 0..3) x (H-half in 0..1). Each core computes
out[b, :, h0:h0+64, :] for h0 = 64*(core%2).

Key algorithmic transform: the [B, 512, H, W] "middle" tensor (masked scatter
of per-region style vectors mu[b,j,:]) is never materialized. Since
middle[b,:,h,w] = mu[b, j*(h,w), :] with j* the last active region,
conv(middle) collapses to a conv over the 5 one-hot region masks sel_j with
per-batch tap tables G[j, cc, tap] = sum_k Wconv[cc, k, tap] * mu[b, j, k].
That turns ~77 GFLOP of 512-channel convs into one K=45 matmul per tile.

The SPADE branch (mask -> shared 3x3 conv -> relu -> gamma/beta 3x3 convs) is
computed directly: shared conv via K=27 im2col, gamma/beta convs as 9
accumulating K=128 taps with gamma and beta fused into one M=128 output.
The sigmoid blending factors are folded into the conv weights and biases.

All conv/table matmuls run in float32r (TF32-like); everything else is fp32.
Each im2col is built by a single multi-dim-AP DMA per output chunk; DMA
issue is spread across the sync/tensor/scalar/gpsimd queues.
"""
import os as _os

import numpy as np

import concourse.bacc as bacc
import concourse.bass as bass
import concourse.mybir as mybir
import concourse.tile as tile
from concourse.bass_utils import run_bass_kernel_spmd

dt = mybir.dt
F32 = dt.float32
F32R = dt.float32 if _os.environ.get("KF32") == "1" else dt.float32r
AF = mybir.ActivationFunctionType
ALU = mybir.AluOpType

B, C, H, W, F, L, NH = 4, 64, 128, 128, 5, 512, 128
GW = 130                    # padded grid width  (image col = grid col - 1)
SR = 66                     # seg/sel/actv grid rows (image row = h0 - 1 + r)
MR = 68                     # mask grid rows (image row = h0 - 2 + r)
SEG_N = SR * GW             # 8580
MASK_N = MR * GW            # 8840
SEG_SZ = SEG_N + 2 * GW + 2 + 520   # sel tail slack for im2col windows
MASK_SZ = MASK_N + 2 * GW + 2 + 390
ROWS = 64                   # output rows per core
NCH = 16                    # main conv chunks (4 rows x 128 cols, N=512)
ACH = 22                    # shared conv chunks (3 rows x 128 cols, N=384)
NCORES = 8


def _win_ap(base_ap, flat):
    """9-tap im2col source view: partitions from base_ap, free dims
    (ty[3] x tx[3] x flat window) as overlapping strided windows."""
    return bass.AP(tensor=base_ap.tensor, offset=base_ap.offset,
                   ap=[base_ap.ap[0], [GW, 3], [1, 3], [1, flat]])


def _build_nc():
    lvl = int(_os.environ.get("KSEC", "8"))
    nc = bacc.Bacc()

    # ---- per-core DRAM inputs -------------------------------------------
    xb = nc.dram_tensor("xb", [C, H * W], F32, kind="ExternalInput")
    xown = nc.dram_tensor("xown", [C, ROWS * W], F32, kind="ExternalInput")
    segg = nc.dram_tensor("segg", [F, SEG_N + 264], F32, kind="ExternalInput")
    maskg = nc.dram_tensor("maskg", [3, MASK_N + 264], F32, kind="ExternalInput")
    codes = nc.dram_tensor("codes", [F, L], F32, kind="ExternalInput")
    fcw = nc.dram_tensor("fcw", [F, L, L], F32, kind="ExternalInput")
    fcbt = nc.dram_tensor("fcbt", [L, F], F32, kind="ExternalInput")
    cgw = nc.dram_tensor("cgw", [C, L * 9], F32, kind="ExternalInput")
    cbw = nc.dram_tensor("cbw", [C, L * 9], F32, kind="ExternalInput")
    sgw = nc.dram_tensor("sgw", [C, NH * 9], F32, kind="ExternalInput")
    sbw = nc.dram_tensor("sbw", [C, NH * 9], F32, kind="ExternalInput")
    ssw = nc.dram_tensor("ssw", [NH, 27], F32, kind="ExternalInput")
    cgb = nc.dram_tensor("cgb", [C, 1], F32, kind="ExternalInput")
    cbb = nc.dram_tensor("cbb", [C, 1], F32, kind="ExternalInput")
    sgbb = nc.dram_tensor("sgbb", [C, 1], F32, kind="ExternalInput")
    sbbb = nc.dram_tensor("sbbb", [C, 1], F32, kind="ExternalInput")
    ssb = nc.dram_tensor("ssb", [NH, 1], F32, kind="ExternalInput")
    bg = nc.dram_tensor("bg", [1, 1], F32, kind="ExternalInput")
    bb = nc.dram_tensor("bb", [1, 1], F32, kind="ExternalInput")
    u5 = nc.dram_tensor("u5", [45, 45], F32, kind="ExternalInput")
    ident = nc.dram_tensor("ident", [128, 128], F32, kind="ExternalInput")
    zz = nc.dram_tensor("zz", [128, 652], F32, kind="ExternalInput")
    hal = nc.dram_tensor("hal", [128, 2], F32, kind="ExternalInput")
    out_d = nc.dram_tensor("out", [C, 8, 1024], F32, kind="ExternalOutput")

    with tile.TileContext(nc) as tc:
        with (
            tc.tile_pool(name="const", bufs=1) as cst,
            tc.tile_pool(name="wcb", bufs=2) as wcbp,
            tc.tile_pool(name="wct", bufs=2) as wctp,
            tc.tile_pool(name="fcwp", bufs=3) as fcwp,
            tc.tile_pool(name="cbcp", bufs=2) as cbcp,
            tc.tile_pool(name="ttp", bufs=2) as ttp,
            tc.tile_pool(name="xs", bufs=2) as xsp,
            tc.tile_pool(name="gb", bufs=3) as gbp,
            tc.tile_pool(name="xn", bufs=2) as xnp,
            tc.tile_pool(name="ot", bufs=2) as otp,
            tc.tile_pool(name="pmain", bufs=2, space="PSUM") as pmain,
            tc.tile_pool(name="paux", bufs=2, space="PSUM") as paux,
            tc.tile_pool(name="gpsp", bufs=3, space="PSUM") as gpsp,
        ):
            # ---- constants / small tiles --------------------------------
            id_t = cst.tile([128, 128], F32)
            nc.sync.dma_start(out=id_t[:], in_=ident[:])
            id_r = cst.tile([128, 128], F32R)
            nc.sync.dma_start(out=id_r[:], in_=ident[:].bitcast(F32R))
            u5r = cst.tile([45, 45], F32R)
            nc.sync.dma_start(out=u5r[:], in_=u5[:].bitcast(F32R))
            ones_t = cst.tile([128, 1], F32)
            nc.gpsimd.memset(ones_t[:], 1.0)
            eps_t = cst.tile([C, 1], F32)
            nc.gpsimd.memset(eps_t[:], 1e-5)
            half1 = cst.tile([128, 1], F32)
            nc.gpsimd.memset(half1[0:64, :], 1.0)
            nc.gpsimd.memset(half1[64:128, :], 0.0)
            zsb = cst.tile([128, 132], F32)
            nc.gpsimd.memset(zsb[:], 0.0)

            # blending factors -> gba (per-cc scale), om_gba (1 - gba)
            graw = cst.tile([128, 1], F32)
            nc.sync.dma_start(out=graw[:], in_=bg[:].to_broadcast((128, 1)))
            braw = cst.tile([128, 1], F32)
            nc.sync.dma_start(out=braw[:], in_=bb[:].to_broadcast((128, 1)))
            gsig = cst.tile([128, 1], F32)
            nc.scalar.activation(gsig[:], graw[:], AF.Sigmoid)
            bsig = cst.tile([128, 1], F32)
            nc.scalar.activation(bsig[:], braw[:], AF.Sigmoid)
            gba = cst.tile([128, 1], F32)
            nc.vector.tensor_copy(gba[0:64, :], gsig[0:64, :])
            nc.vector.tensor_copy(gba[64:128, :], bsig[64:128, :])
            om_gba = cst.tile([128, 1], F32)
            nc.scalar.activation(om_gba[:], gba[:], AF.Identity, bias=ones_t[:], scale=-1.0)

            # biases: rows 0:64 gamma, 64:128 beta
            convb = cst.tile([128, 1], F32)
            nc.sync.dma_start(out=convb[0:64, :], in_=cgb[:])
            nc.sync.dma_start(out=convb[64:128, :], in_=cbb[:])
            spadeb = cst.tile([128, 1], F32)
            nc.sync.dma_start(out=spadeb[0:64, :], in_=sgbb[:])
            nc.sync.dma_start(out=spadeb[64:128, :], in_=sbbb[:])
            ssb_t = cst.tile([NH, 1], F32)
            nc.sync.dma_start(out=ssb_t[:], in_=ssb[:])
            hal_t = cst.tile([128, 2], F32)
            nc.sync.dma_start(out=hal_t[:], in_=hal[:])

            tb1 = cst.tile([128, 1], F32)
            nc.vector.tensor_mul(tb1[:], convb[:], gba[:])
            tb2 = cst.tile([128, 1], F32)
            nc.vector.tensor_mul(tb2[:], spadeb[:], om_gba[:])
            bias_t = cst.tile([128, 1], F32)
            nc.vector.tensor_add(bias_t[:], tb1[:], tb2[:])
            bias1_t = cst.tile([128, 1], F32)
            nc.vector.tensor_add(bias1_t[:], bias_t[:], half1[:])

            # ---- grids: pre-shifted replicated loads --------------------
            # sel45 rows (r=(ty,tx), j) = seg_j shifted by ty*GW+tx; sel
            # overwrites seg in place. mask27 rows (r, ch) likewise.
            sel45 = cst.tile([45, SEG_N], F32R)
            segp = segg[:].ap[0][0]
            for ty in range(3):
                src = bass.AP(tensor=segg[:].tensor, offset=ty * GW,
                              ap=[[1, 3], [segp, F], [1, SEG_N]])
                nc.sync.dma_start(out=sel45[15 * ty:15 * ty + 15, :],
                                  in_=src.bitcast(F32R))
            mask27 = cst.tile([27, MASK_N], F32R)
            maskp_ = maskg[:].ap[0][0]
            for ty in range(3):
                src = bass.AP(tensor=maskg[:].tensor, offset=ty * GW,
                              ap=[[1, 3], [maskp_, 3], [1, MASK_N]])
                nc.scalar.dma_start(out=mask27[9 * ty:9 * ty + 9, :],
                                    in_=src.bitcast(F32R))

            # ---- region masks: cnt -> t -> sel (in place over seg) ------
            off = 0 if lvl >= 2 else SEG_N
            while off < SEG_N:
                n = min(512, SEG_N - off)
                pc = paux.tile([45, 512], F32, tag="aux")
                nc.tensor.matmul(pc[:, 0:n], u5r[:], sel45[:, off:off + n],
                                 start=True, stop=True)
                pt5 = paux.tile([45, 512], F32, tag="aux")
                nc.scalar.activation(pt5[:, 0:n], pc[:, 0:n], AF.Relu,
                                     bias=ones_t[0:45, :], scale=-1.0)
                nc.vector.tensor_mul(sel45[:, off:off + n],
                                     sel45[:, off:off + n].bitcast(F32),
                                     pt5[:, 0:n])
                off += n

            # ---- shared conv (mask 3 -> NH), K=27 pre-shifted rows ------
            if lvl >= 3:
                sswf = cst.tile([NH, 27], F32)
                nc.sync.dma_start(out=sswf[:], in_=ssw[:])
                ptp = paux.tile([27, 128], F32, tag="aux")
                nc.tensor.transpose(ptp[:], sswf[:], id_t[:])
                sswT = cst.tile([27, 128], F32R)
                nc.scalar.activation(sswT[:], ptp[:], AF.Copy)

                actv = cst.tile([NH, SR, GW], F32R)
                bord = actv[:, :, 0:1]
                nc.vector.tensor_copy(
                    bass.AP(tensor=bord.tensor, offset=bord.offset,
                            ap=[bord.ap[0], [GW, SR], [GW - 1, 2]]),
                    zsb[:].rearrange("p (a b) -> p a b", a=SR))
                m3 = mask27[:].rearrange("p (r c) -> p r c", c=GW)
                for a in range(ACH):
                    r = 3 * a
                    psh = paux.tile([NH, 3, 128], F32, tag="aux")
                    nc.tensor.matmul(psh[:], sswT[:], m3[:, r:r + 3, 0:128],
                                     start=True, stop=True)
                    nc.scalar.activation(actv[:, r:r + 3, 1:129], psh[:], AF.Relu,
                                         bias=ssb_t[:], scale=1.0)
                # out-of-image actv rows are conv2d zero-padding
                nc.vector.tensor_scalar_mul(actv[:, 0, :], actv[:, 0, :].bitcast(F32),
                                            hal_t[:, 0:1])
                nc.vector.tensor_scalar_mul(actv[:, SR - 1, :], actv[:, SR - 1, :].bitcast(F32),
                                            hal_t[:, 1:2])

            # ---- spade gamma/beta lhsT ----------------------------------
            if lvl >= 6:
                sgb = cst.tile([128, 1152], F32)
                nc.sync.dma_start(out=sgb[0:64, :], in_=sgw[:])
                nc.sync.dma_start(out=sgb[64:128, :], in_=sbw[:])
                nc.vector.tensor_scalar_mul(sgb[:], sgb[:], om_gba[:])
                spT = cst.tile([128, 9, 128], F32R)
                sgb3 = sgb[:].rearrange("p (l t) -> p l t", t=9)
                for t in range(9):
                    pt = paux.tile([128, 128], F32, tag="aux")
                    nc.tensor.transpose(pt[:], sgb3[:, :, t], id_t[:])
                    nc.scalar.activation(spT[:, t, :], pt[:], AF.Copy)

            # ---- mu path: z[j,k] = sum_l fcw[j,k,l]*c[j,l]; mu = relu(z+b)
            if lvl >= 4:
                fcbt_sb = cst.tile([128, 4, F], F32)
                for kb in range(4):
                    nc.sync.dma_start(out=fcbt_sb[:, kb, :],
                                      in_=fcbt[kb * 128:(kb + 1) * 128, :])
                z_sb = cst.tile([128, 4, F], F32)
                muT = cst.tile([128, 4, F], F32R)
                for j in range(F):
                    cbc = cbcp.tile([128, L], F32, tag="cbc")
                    nc.scalar.dma_start(out=cbc[:],
                                        in_=codes[j:j + 1, :].to_broadcast((128, L)))
                    eng = nc.vector if j < 3 else nc.gpsimd
                    for kb in range(4):
                        fw = fcwp.tile([128, L], F32, tag="fcw")
                        nc.sync.dma_start(out=fw[:], in_=fcw[j, kb * 128:(kb + 1) * 128, :])
                        tts = ttp.tile([128, L], F32, tag="tts")
                        eng.tensor_mul(tts[:], fw[:], cbc[:])
                        nc.vector.reduce_sum(out=z_sb[:, kb, j:j + 1], in_=tts[:],
                                             axis=mybir.AxisListType.X)
                for kb in range(4):
                    nc.vector.tensor_add(z_sb[:, kb, :], z_sb[:, kb, :],
                                         fcbt_sb[:, kb, :])
                for kb in range(4):
                    nc.scalar.activation(muT[:, kb, :], z_sb[:, kb, :], AF.Relu)

            # ---- conv gamma/beta tap tables G -> selG (rows t*5+j) ------
            if lvl >= 5:
                gps = [gpsp.tile([F, 3, 128], F32, tag="gps", name=f"gps{_g}")
                       for _g in range(3)]
                for kb in range(4):
                    wcb = wcbp.tile([128, 1152], F32, tag="wcb")
                    nc.sync.dma_start(out=wcb[0:64, :], in_=cgw[:, kb * 1152:(kb + 1) * 1152])
                    nc.sync.dma_start(out=wcb[64:128, :], in_=cbw[:, kb * 1152:(kb + 1) * 1152])
                    nc.vector.tensor_scalar_mul(wcb[:], wcb[:], gba[:])
                    wct = wctp.tile([128, 9, 128], F32R, tag="wct")
                    wcb3 = wcb[:].rearrange("p (l t) -> p l t", t=9)
                    for t in range(9):
                        pt = paux.tile([128, 128], F32, tag="aux")
                        nc.tensor.transpose(pt[:], wcb3[:, :, t], id_t[:])
                        nc.scalar.activation(wct[:, t, :], pt[:], AF.Copy)
                    for g in range(3):
                        nc.tensor.matmul(gps[g][:], muT[:, kb, :], wct[:, 3 * g:3 * g + 3, :],
                                         start=(kb == 0), stop=(kb == 3))
                selG = cst.tile([45, 128], F32R)
                gstage = cst.tile([F, 9, 128], F32)
                for g in range(3):
                    nc.scalar.activation(gstage[:, 3 * g:3 * g + 3, :], gps[g][:], AF.Copy)
                for t in range(9):
                    nc.sync.dma_start(out=selG[F * t:F * t + F, :],
                                      in_=gstage[:, t, :].bitcast(F32R))

            # ---- instance-norm stats over the full plane ----------------
            if lvl >= 7:
                stats_t = cst.tile([C, 32, 6], F32)
                for q in range(8):
                    xt = xsp.tile([C, 4, 512], F32, tag="xs")
                    nc.scalar.dma_start(out=xt[:], in_=xb[:, q * 2048:(q + 1) * 2048]
                                        .rearrange("c (k n) -> c k n", k=4))
                    for k in range(4):
                        nc.vector.bn_stats(out=stats_t[:, 4 * q + k, :], in_=xt[:, k, :])
                mv = cst.tile([C, 2], F32)
                nc.vector.bn_aggr(out=mv[:], in_=stats_t[:])
                sd = cst.tile([C, 1], F32)
                nc.scalar.activation(sd[:], mv[:, 1:2], AF.Sqrt, bias=eps_t[:], scale=1.0)
                rstd = cst.tile([C, 1], F32)
                nc.vector.reciprocal(rstd[:], sd[:])
                nbias = cst.tile([C, 1], F32)
                nc.vector.tensor_mul(nbias[:], mv[:, 0:1], rstd[:])
                nc.vector.tensor_scalar_mul(nbias[:], nbias[:], -1.0)

            # ---- main conv + epilogue, 16 chunks of 4 rows --------------
            if lvl >= 8:
                xt2 = None
                xnt = None
                s3 = sel45[:].rearrange("p (r c) -> p r c", c=GW)
                for i in range(NCH):
                    if i % 2 == 0:
                        xt2 = xnp.tile([C, 2, 4, 128], F32, tag="xn")
                        nc.gpsimd.dma_start(out=xt2[:],
                                            in_=xown[:, i * 512:(i + 2) * 512].rearrange(
                                                "c (k r w) -> c k r w", k=2, r=4))
                        xnt = otp.tile([C, 2, 4, 128], F32, tag="ot")
                    pm = pmain.tile([128, 4, 128], F32, tag="pm")
                    for t in range(9):
                        ty, tx = divmod(t, 3)
                        nc.tensor.matmul(pm[:], spT[:, t, :],
                                         actv[:, 4 * i + ty:4 * i + ty + 4, tx:tx + 128],
                                         start=(t == 0), stop=False)
                    nc.tensor.matmul(pm[:], selG[:], s3[:, 4 * i:4 * i + 4, 0:128],
                                     start=False, stop=True)

                    gb = gbp.tile([128, 4, 128], F32R, tag="gb")
                    nc.scalar.activation(gb[:], pm[:], AF.Identity,
                                         bias=bias1_t[:], scale=1.0)
                    # move beta rows 64:128 down to partitions 0:64 via PE
                    pb = gpsp.tile([64, 4, 128], F32, tag="gps", name="pb")
                    nc.tensor.matmul(pb[:].rearrange("p t c -> p (t c)"), id_r[:, 64:128],
                                     gb[:].rearrange("p t c -> p (t c)"),
                                     start=True, stop=True)
                    k = i % 2
                    nc.gpsimd.tensor_scalar(xnt[:, k, :, :], xt2[:, k, :, :],
                                            rstd[:], nbias[:],
                                            op0=ALU.mult, op1=ALU.add)
                    nc.vector.tensor_mul(xnt[:, k, :, :], xnt[:, k, :, :],
                                         gb[0:64, :, :].bitcast(F32))
                    nc.vector.tensor_add(xnt[:, k, :, :].rearrange("p t c -> p (t c)"),
                                         xnt[:, k, :, :].rearrange("p t c -> p (t c)"),
                                         pb[:].rearrange("p t c -> p (t c)"))
                    if i % 2 == 1:
                        nc.sync.dma_start(out=out_d[:, i // 2, :],
                                          in_=xnt[:].rearrange("c k r w -> c (k r w)"))

    nc.finalize()
    return nc


_NC = None


def kernel(**inputs):
    global _NC
    x = np.asarray(inputs["x"], dtype=np.float32)
    segmap = np.asarray(inputs["segmap"], dtype=np.float32)
    codes_vector = np.asarray(inputs["codes_vector"], dtype=np.float32)
    mask = np.asarray(inputs["mask"], dtype=np.float32)
    fc_w = np.ascontiguousarray(np.asarray(inputs["fc_w"], dtype=np.float32))
    fc_b = np.asarray(inputs["fc_b"], dtype=np.float32)
    conv_gamma_w = np.asarray(inputs["conv_gamma_w"], dtype=np.float32)
    conv_gamma_b = np.asarray(inputs["conv_gamma_b"], dtype=np.float32)
    conv_beta_w = np.asarray(inputs["conv_beta_w"], dtype=np.float32)
    conv_beta_b = np.asarray(inputs["conv_beta_b"], dtype=np.float32)
    spade_shared_w = np.asarray(inputs["spade_shared_w"], dtype=np.float32)
    spade_shared_b = np.asarray(inputs["spade_shared_b"], dtype=np.float32)
    spade_gamma_w = np.asarray(inputs["spade_gamma_w"], dtype=np.float32)
    spade_gamma_b = np.asarray(inputs["spade_gamma_b"], dtype=np.float32)
    spade_beta_w = np.asarray(inputs["spade_beta_w"], dtype=np.float32)
    spade_beta_b = np.asarray(inputs["spade_beta_b"], dtype=np.float32)
    blending_gamma = np.asarray(inputs["blending_gamma"], dtype=np.float32)
    blending_beta = np.asarray(inputs["blending_beta"], dtype=np.float32)

    if _NC is None:
        _NC = _build_nc()

    shared = {
        "fcw": np.ascontiguousarray(fc_w),
        "fcbt": np.ascontiguousarray(fc_b.T),
        "cgw": np.ascontiguousarray(conv_gamma_w.reshape(C, L * 9)),
        "cbw": np.ascontiguousarray(conv_beta_w.reshape(C, L * 9)),
        "sgw": np.ascontiguousarray(spade_gamma_w.reshape(C, NH * 9)),
        "sbw": np.ascontiguousarray(spade_beta_w.reshape(C, NH * 9)),
        "ssw": np.ascontiguousarray(spade_shared_w.transpose(0, 2, 3, 1).reshape(NH, 27)),
        "cgb": conv_gamma_b.reshape(C, 1), "cbb": conv_beta_b.reshape(C, 1),
        "sgbb": spade_gamma_b.reshape(C, 1), "sbbb": spade_beta_b.reshape(C, 1),
        "ssb": spade_shared_b.reshape(NH, 1),
        "bg": blending_gamma.reshape(1, 1), "bb": blending_beta.reshape(1, 1),
        "u5": np.kron(np.eye(9, dtype=np.float32), np.tril(np.ones((F, F), np.float32), -1)),
        "ident": np.eye(128, dtype=np.float32),
        "zz": np.zeros((128, 652), np.float32),
    }

    in_maps = []
    for c in range(NCORES):
        b, half = divmod(c, 2)
        h0 = half * ROWS
        segp = np.zeros((F, SR * GW + 264), np.float32).reshape(F, -1)
        segp2 = np.zeros((F, SR, GW), np.float32)
        r_lo, r_hi = h0 - 1, h0 + ROWS + 1  # exclusive
        s_lo, s_hi = max(r_lo, 0), min(r_hi, H)
        segp2[:, s_lo - r_lo:s_hi - r_lo, 1:129] = segmap[b, :, s_lo:s_hi, :]
        segp[:, 0:SR * GW] = segp2.reshape(F, -1)
        maskp = np.zeros((3, MR * GW + 264), np.float32)
        maskp2 = np.zeros((3, MR, GW), np.float32)
        m_lo, m_hi = h0 - 2, h0 + ROWS + 2
        ms_lo, ms_hi = max(m_lo, 0), min(m_hi, H)
        maskp2[:, ms_lo - m_lo:ms_hi - m_lo, 1:129] = mask[b, :, ms_lo:ms_hi, :]
        maskp[:, 0:MR * GW] = maskp2.reshape(3, -1)
        in_maps.append(dict(
            shared,
            xb=np.ascontiguousarray(x[b].reshape(C, H * W)),
            xown=np.ascontiguousarray(x[b, :, h0:h0 + ROWS, :].reshape(C, ROWS * W)),
            hal=np.ones((128, 2), np.float32) * np.array(
                [0.0 if h0 == 0 else 1.0, 0.0 if h0 + ROWS == H else 1.0],
                np.float32)[None, :],
            segg=np.ascontiguousarray(segp),
            maskg=np.ascontiguousarray(maskp),
            codes=np.ascontiguousarray(codes_vector[b]),
        ))

    res = run_bass_kernel_spmd(_NC, in_maps, list(range(NCORES)))

    out = np.empty((B, C, H, W), np.float32)
    for c in range(NCORES):
        b, half = divmod(c, 2)
        h0 = half * ROWS
        out[b, :, h0:h0 + ROWS, :] = res.results[c]["out"].reshape(C, ROWS, W)
    return out
